# revision 14
# baseline (speedup 1.0000x reference)
"""CenterNet decode + pseudo-NMS + top-K for Trainium2 (8 NeuronCores).

Observation: the reference only returns results for batch element 0
(`topk_scores[0]`, `topk_clses[0]`, and boxes gathered with `topk_inds[0]`),
so only `cls_pred[0]` / `txty_pred[0]` / `twth_pred[0]` influence the output.
Working on raw logits (sigmoid is strictly monotone) keeps ordering and the
5x5-peak test identical while avoiding any dense transcendentals.

Device (classes of batch 0 sharded 10-per-core across 8 cores):
  - stream each class heatmap [256,256] into SBUF as [128, 512]
    (partition p holds image rows 2p, 2p+1), one DMA per class,
    alternating between the two HWDGE rings (SP + Activation)
  - 2x8 block-max pool -> cx [128, 320] per core (DVE tensor_reduce)
  - one vector.max + max_index pass extracts the top-8 block maxima per
    partition (values + block positions)

Host epilogue (O(candidates), numpy):
  - every extracted 2x8 block is re-examined against the raw heatmap:
    within each aligned 2x2 sub-block only the argmax can survive the 5x5
    peak test (two surviving peaks are >=3 apart in Chebyshev distance),
    so 4 exact peak tests per block recover every peak in extracted blocks
  - global sort of verified peaks -> top-100 -> sigmoid + box decode at
    the 100 winning positions only
  - soundness check: any peak NOT recovered lives in a non-extracted
    block, whose block-max is <= the partition's 8th-largest value
    vals[p,7]; if some vals[p,7] >= the 100th-best score (or an extracted
    index repeats due to an exact value tie), fall back to an exact numpy
    implementation. With continuous random inputs this never triggers.
"""

import numpy as np

NCORES = 8
CPC = 10  # classes per core
H = W = 256
BW_ = 8  # block width (block = 2 rows x 8 cols)
NBLK = W // BW_  # 32 blocks per class per partition-row
CXW = CPC * NBLK  # 320
NCAND = 8
TOPK = 100
STRIDE = 4

_CACHE = {}


def _build_bass():
    if "nc" in _CACHE:
        return _CACHE["nc"]
    import concourse.bass as bass
    import concourse.mybir as mybir

    # Bass.__init__ ends with an all-engine barrier; the PE engine takes
    # ~3us to come up, so every other engine idles at that barrier before
    # the first DMA can issue. This kernel never touches PE/const-APs and
    # every cross-engine dependency in the body is explicitly semaphore-
    # guarded, so the init barrier is safely elidable (the Block-exit
    # barrier still synchronizes everything at the end).
    _orig_aeb = bass.Bass.all_engine_barrier
    bass.Bass.all_engine_barrier = lambda self, **kw: None
    try:
        nc = bass.Bass(trn_type="TRN2")
    finally:
        bass.Bass.all_engine_barrier = _orig_aeb
    x = nc.dram_tensor("x", [CPC, H, W], mybir.dt.float32, kind="ExternalInput")
    # packed output: [:, :8] = top-8 values (f32), [:, 8:] = indices (as f32)
    opack = nc.dram_tensor(
        "opack", [128, 2 * NCAND], mybir.dt.float32, kind="ExternalOutput"
    )

    xt = [
        nc.alloc_sbuf_tensor(f"xt{c}", [128, 512], mybir.dt.float32)
        for c in range(CPC)
    ]
    cx = nc.alloc_sbuf_tensor("cx", [128, CXW], mybir.dt.float32)
    packed = nc.alloc_sbuf_tensor("packed", [128, 2 * NCAND], mybir.dt.float32)
    idx = nc.alloc_sbuf_tensor("idx", [128, NCAND], mybir.dt.uint32)
    # one completion sem per input DMA (a shared cumulative sem would be
    # unsound: 16 SDMA engines complete independently, so sem >= 16*g does
    # not imply the g-th transfer fully landed while later ones are queued)
    gsem = [nc.alloc_semaphore(f"gsem{c}") for c in range(CPC)]
    vsem = nc.alloc_semaphore("vsem")
    osem = nc.alloc_semaphore("osem")

    def dma_in(eng, c):
        eng.dma_start(
            out=xt[c][:].rearrange("p (a w) -> p a w", a=2),
            in_=x[c].rearrange("(p a) w -> p a w", a=2),
        ).then_inc(gsem[c], 16)

    with nc.Block() as block:

        @block.sync
        def _(sync):
            for c in range(0, CPC, 2):  # even classes on the SP HWDGE ring
                dma_in(sync, c)
            sync.wait_ge(vsem, 1)
            sync.dma_start(out=opack[:], in_=packed[:]).then_inc(osem, 16)

        @block.scalar
        def _(scalar):
            for c in range(1, CPC, 2):  # odd classes on the ACT HWDGE ring
                dma_in(scalar, c)

        @block.vector
        def _(vector):
            for c in range(CPC):
                vector.wait_ge(gsem[c], 16)
                # 2x8 block max: [p, j, a, b] -> [p, j]
                vector.reduce_max(
                    out=cx[:, c * NBLK : (c + 1) * NBLK],
                    in_=xt[c][:].rearrange("p (a j b) -> p j a b", a=2, b=BW_),
                    axis=mybir.AxisListType.XY,
                )
            # DVE ops overlap within the engine: drain between dependent ops
            vector.drain()  # reduces wrote cx
            v8 = packed[:, 0:NCAND]
            vector.max(out=v8, in_=cx[:])
            vector.drain()  # max wrote v8, read by max_index
            vector.max_index(out=idx[:], in_max=v8, in_values=cx[:])
            vector.drain()  # idx settled before the cast below
            # cast indices into the packed f32 tile (exact: values < 2^24)
            vector.tensor_copy(packed[:, NCAND : 2 * NCAND], idx[:])
            vector.drain().then_inc(vsem, 1)

    # wait for the output DMA's completion receipt after the block-exit
    # barrier, so the barrier overlaps the DMA flight instead of serializing
    nc.sync.wait_ge(osem, 16)

    _CACHE["nc"] = nc
    return nc


def _run_device(cls0, trace=False, **trace_kwargs):
    """cls0: np.float32 [80,256,256] -> (vals [8,128,8], idx [8,128,8], results)"""
    from concourse.bass_utils import run_bass_kernel_spmd

    nc = _build_bass()
    in_maps = [
        {"x": np.ascontiguousarray(cls0[c * CPC : (c + 1) * CPC])}
        for c in range(NCORES)
    ]
    res = run_bass_kernel_spmd(
        nc, in_maps, core_ids=list(range(NCORES)), trace=trace, **trace_kwargs
    )
    packed = np.stack([r["opack"] for r in res.results])
    vals = packed[:, :, :NCAND]
    idx = packed[:, :, NCAND:].astype(np.uint32)
    return vals, idx, res


def _postprocess(cls0, txty0, twth0, vals, idx):
    """Host epilogue. cls0 [80,256,256] f32, txty0/twth0 [2,256,256] f32,
    vals/idx [8,128,8]. Returns (bbox [100,4] f32, scores [100] f32, clses [100] i32).
    """
    core = np.arange(NCORES)[:, None, None]
    part = np.arange(128)[None, :, None]
    f = idx.reshape(-1).astype(np.int64)
    cls = (core * CPC + (idx // NBLK)).reshape(-1).astype(np.int64)
    j = (f % NBLK).astype(np.int64)
    p = np.broadcast_to(part, idx.shape).reshape(-1).astype(np.int64)

    # extracted 2x8 blocks -> 4 aligned 2x2 sub-blocks; only a sub-block
    # argmax can be a 5x5 peak (surviving peaks are >=3 apart)
    blk = cls0[
        cls[:, None, None],
        (2 * p)[:, None, None] + np.arange(2)[None, :, None],
        (BW_ * j)[:, None, None] + np.arange(BW_)[None, None, :],
    ]  # [N, 2, 8]
    N = blk.shape[0]
    sub = blk.reshape(N, 2, 4, 2).transpose(0, 2, 1, 3).reshape(N, 4, 4)  # [N,4,(2x2)]
    pos = sub.argmax(axis=2)  # [N, 4] in 0..3
    v = np.take_along_axis(sub, pos[:, :, None], axis=2)[:, :, 0]  # [N, 4]
    y = 2 * p[:, None] + pos // 2
    x = BW_ * j[:, None] + 2 * np.arange(4)[None, :] + pos % 2
    cls4 = np.broadcast_to(cls[:, None], (N, 4))
    v, y, x, cls4 = v.ravel(), y.ravel(), x.ravel(), cls4.ravel()

    # exact 5x5 peak test (clipped window == -inf padding for max)
    dy = np.arange(-2, 3)
    yy = np.clip(y[:, None] + dy[None, :], 0, H - 1)
    xx = np.clip(x[:, None] + dy[None, :], 0, W - 1)
    win = cls0[cls4[:, None, None], yy[:, :, None], xx[:, None, :]]
    is_peak = win.max(axis=(1, 2)) == v

    vv, cc, yy_, xx_ = v[is_peak], cls4[is_peak], y[is_peak], x[is_peak]
    order = np.argsort(-vv, kind="stable")[:TOPK]
    ok = order.size == TOPK
    if ok:
        vv, cc, yy_, xx_ = vv[order], cc[order], yy_[order], xx_[order]
        thr = vv[-1]
        # soundness: nothing outside the extracted blocks can reach the
        # top-100, and no extracted slot was lost to an exact value tie
        if (vals[:, :, -1] >= thr).any():
            ok = False
        else:
            srt = np.sort(idx, axis=2)
            dup = srt[:, :, 1:] == srt[:, :, :-1]
            if dup.any():
                dupv = np.minimum(vals[:, :, 1:], vals[:, :, :-1])[dup]
                if (dupv >= thr).any():
                    ok = False
    if not ok:  # pragma: no cover - never expected with continuous inputs
        return _reference_numpy(cls0, txty0, twth0)

    scores = (1.0 / (1.0 + np.exp(-vv))).astype(np.float32)
    clses = cc.astype(np.int32)

    sig = lambda a: (1.0 / (1.0 + np.exp(-a.astype(np.float32)))).astype(np.float32)
    cxp = (sig(txty0[0, yy_, xx_]) + xx_.astype(np.float32)) * STRIDE
    cyp = (sig(txty0[1, yy_, xx_]) + yy_.astype(np.float32)) * STRIDE
    wp = np.exp(twth0[0, yy_, xx_].astype(np.float32)) * STRIDE
    hp = np.exp(twth0[1, yy_, xx_].astype(np.float32)) * STRIDE
    scale = np.float32(H * STRIDE)
    bbox = (
        np.stack([cxp - wp * 0.5, cyp - hp * 0.5, cxp + wp * 0.5, cyp + hp * 0.5], -1)
        / scale
    )
    bbox = np.clip(bbox, 0.0, 1.0).astype(np.float32)
    return bbox, scores, clses


def _reference_numpy(cls0, txty0, twth0):
    """Exact safety-net implementation (numpy only); never expected to run."""
    prob = (1.0 / (1.0 + np.exp(-cls0.astype(np.float64)))).astype(np.float32)
    pad = np.full((prob.shape[0], H + 4, W + 4), -np.inf, np.float32)
    pad[:, 2:-2, 2:-2] = prob
    hmax = prob.copy()
    for ddy in range(5):
        for ddx in range(5):
            np.maximum(hmax, pad[:, ddy : ddy + H, ddx : ddx + W], out=hmax)
    masked = prob * (hmax == prob)
    C = masked.shape[0]
    flat = masked.reshape(C, H * W)
    i1 = np.argsort(-flat, axis=1, kind="stable")[:, :TOPK]
    s1 = np.take_along_axis(flat, i1, axis=1)
    s1f = s1.reshape(-1)
    i2 = np.argsort(-s1f, kind="stable")[:TOPK]
    clses = (i2 // TOPK).astype(np.int32)
    inds = i1.reshape(-1)[i2]
    yy_, xx_ = inds // W, inds % W
    scores = s1f[i2].astype(np.float32)
    sig = lambda a: (1.0 / (1.0 + np.exp(-a.astype(np.float32)))).astype(np.float32)
    cxp = (sig(txty0[0, yy_, xx_]) + xx_.astype(np.float32)) * STRIDE
    cyp = (sig(txty0[1, yy_, xx_]) + yy_.astype(np.float32)) * STRIDE
    wp = np.exp(twth0[0, yy_, xx_].astype(np.float32)) * STRIDE
    hp = np.exp(twth0[1, yy_, xx_].astype(np.float32)) * STRIDE
    scale = np.float32(H * STRIDE)
    bbox = (
        np.stack([cxp - wp * 0.5, cyp - hp * 0.5, cxp + wp * 0.5, cyp + hp * 0.5], -1)
        / scale
    )
    return np.clip(bbox, 0.0, 1.0).astype(np.float32), scores, clses


def kernel(cls_pred, txty_pred, twth_pred):
    cls0 = np.ascontiguousarray(np.asarray(cls_pred[0], dtype=np.float32))
    txty0 = np.asarray(txty_pred[0], dtype=np.float32)
    twth0 = np.asarray(twth_pred[0], dtype=np.float32)
    vals, idx, _ = _run_device(cls0)
    return _postprocess(cls0, txty0, twth0, vals, idx)


# revision 15
# speedup vs baseline: 1.0437x; 1.0437x over previous
"""CenterNet decode + pseudo-NMS + top-K for Trainium2 (8 NeuronCores).

Observation: the reference only returns results for batch element 0
(`topk_scores[0]`, `topk_clses[0]`, and boxes gathered with `topk_inds[0]`),
so only `cls_pred[0]` / `txty_pred[0]` / `twth_pred[0]` influence the output.
Working on raw logits (sigmoid is strictly monotone) keeps ordering and the
5x5-peak test identical while avoiding any dense transcendentals.

Device (classes of batch 0 sharded 10-per-core across 8 cores):
  - stream each class heatmap [256,256] into SBUF as [128, 512]
    (partition p holds image rows 2p, 2p+1), one DMA per class,
    alternating between the two HWDGE rings (SP + Activation)
  - 2x8 block-max pool -> cx [128, 320] per core (DVE tensor_reduce)
  - one vector.max + max_index pass extracts the top-8 block maxima per
    partition (values + block positions)

Host epilogue (O(candidates), numpy):
  - every extracted 2x8 block is re-examined against the raw heatmap:
    within each aligned 2x2 sub-block only the argmax can survive the 5x5
    peak test (two surviving peaks are >=3 apart in Chebyshev distance),
    so 4 exact peak tests per block recover every peak in extracted blocks
  - global sort of verified peaks -> top-100 -> sigmoid + box decode at
    the 100 winning positions only
  - soundness check: any peak NOT recovered lives in a non-extracted
    block, whose block-max is <= the partition's 8th-largest value
    vals[p,7]; if some vals[p,7] >= the 100th-best score (or an extracted
    index repeats due to an exact value tie), fall back to an exact numpy
    implementation. With continuous random inputs this never triggers.
"""

import numpy as np

NCORES = 8
CPC = 10  # classes per core
H = W = 256
BW_ = 8  # block width (block = 2 rows x 8 cols)
NBLK = W // BW_  # 32 blocks per class per partition-row
CXW = CPC * NBLK  # 320
NCAND = 8
TOPK = 100
STRIDE = 4

_CACHE = {}


def _build_bass():
    if "nc" in _CACHE:
        return _CACHE["nc"]
    import concourse.bass as bass
    import concourse.mybir as mybir

    # Bass.__init__ ends with an all-engine barrier; the PE engine takes
    # ~3us to come up, so every other engine idles at that barrier before
    # the first DMA can issue. This kernel never touches PE/const-APs and
    # every cross-engine dependency in the body is explicitly semaphore-
    # guarded, so the init barrier is safely elidable (the Block-exit
    # barrier still synchronizes everything at the end).
    _orig_aeb = bass.Bass.all_engine_barrier
    bass.Bass.all_engine_barrier = lambda self, **kw: None
    try:
        nc = bass.Bass(trn_type="TRN2")
    finally:
        bass.Bass.all_engine_barrier = _orig_aeb
    x = nc.dram_tensor("x", [CPC, H, W], mybir.dt.float32, kind="ExternalInput")
    # packed output: [:, :8] = top-8 values (f32), [:, 8:] = indices (as f32)
    opack = nc.dram_tensor(
        "opack", [128, 2 * NCAND], mybir.dt.float32, kind="ExternalOutput"
    )

    xt = nc.alloc_sbuf_tensor("xt", [128, CPC * 512], mybir.dt.float32)
    cx = nc.alloc_sbuf_tensor("cx", [128, CXW], mybir.dt.float32)
    packed = nc.alloc_sbuf_tensor("packed", [128, 2 * NCAND], mybir.dt.float32)
    idx = nc.alloc_sbuf_tensor("idx", [128, NCAND], mybir.dt.uint32)

    # input plan: (ring, class_lo, class_hi) — small first chunks for a fast
    # pipeline start, 2-class chunks in the middle for bandwidth, small last
    # chunks so the final reduce isn't gated on a large transfer
    PLAN = [(0, 0, 1), (1, 1, 2), (0, 2, 4), (1, 4, 6), (0, 6, 8), (1, 8, 9), (0, 9, 10)]

    # one completion sem per input DMA (a shared cumulative sem would be
    # unsound: 16 SDMA engines complete independently, so sem >= 16*g does
    # not imply the g-th transfer fully landed while later ones are queued)
    gsem = [nc.alloc_semaphore(f"gsem{i}") for i in range(len(PLAN))]
    vsem = nc.alloc_semaphore("vsem")
    osem = nc.alloc_semaphore("osem")
    cls_sem = {}  # class -> (sem, transfer idx)
    for i, (_, c0, c1) in enumerate(PLAN):
        for c in range(c0, c1):
            cls_sem[c] = i

    def dma_in(eng, i):
        _, c0, c1 = PLAN[i]
        eng.dma_start(
            out=xt[:, c0 * 512 : c1 * 512].rearrange(
                "p (c a w) -> p c a w", c=c1 - c0, a=2
            ),
            in_=x[c0:c1].rearrange("c (p a) w -> p c a w", a=2),
        ).then_inc(gsem[i], 16)

    with nc.Block() as block:

        @block.sync
        def _(sync):
            for i, (ring, _, _) in enumerate(PLAN):
                if ring == 0:
                    dma_in(sync, i)
            sync.wait_ge(vsem, 1)
            sync.dma_start(out=opack[:], in_=packed[:]).then_inc(osem, 16)

        @block.scalar
        def _(scalar):
            for i, (ring, _, _) in enumerate(PLAN):
                if ring == 1:
                    dma_in(scalar, i)

        @block.vector
        def _(vector):
            waited = set()
            for c in range(CPC):
                if cls_sem[c] not in waited:
                    waited.add(cls_sem[c])
                    vector.wait_ge(gsem[cls_sem[c]], 16)
                # 2x8 block max: [p, j, a, b] -> [p, j]
                vector.reduce_max(
                    out=cx[:, c * NBLK : (c + 1) * NBLK],
                    in_=xt[:, c * 512 : (c + 1) * 512].rearrange(
                        "p (a j b) -> p j a b", a=2, b=BW_
                    ),
                    axis=mybir.AxisListType.XY,
                )
            # DVE ops overlap within the engine: drain between dependent ops
            vector.drain()  # reduces wrote cx
            v8 = packed[:, 0:NCAND]
            vector.max(out=v8, in_=cx[:])
            vector.drain()  # max wrote v8, read by max_index
            vector.max_index(out=idx[:], in_max=v8, in_values=cx[:])
            vector.drain()  # idx settled before the cast below
            # cast indices into the packed f32 tile (exact: values < 2^24)
            vector.tensor_copy(packed[:, NCAND : 2 * NCAND], idx[:])
            vector.drain().then_inc(vsem, 1)

    # wait for the output DMA's completion receipt after the block-exit
    # barrier, so the barrier overlaps the DMA flight instead of serializing
    nc.sync.wait_ge(osem, 16)

    _CACHE["nc"] = nc
    return nc


def _run_device(cls0, trace=False, **trace_kwargs):
    """cls0: np.float32 [80,256,256] -> (vals [8,128,8], idx [8,128,8], results)"""
    from concourse.bass_utils import run_bass_kernel_spmd

    nc = _build_bass()
    in_maps = [
        {"x": np.ascontiguousarray(cls0[c * CPC : (c + 1) * CPC])}
        for c in range(NCORES)
    ]
    res = run_bass_kernel_spmd(
        nc, in_maps, core_ids=list(range(NCORES)), trace=trace, **trace_kwargs
    )
    packed = np.stack([r["opack"] for r in res.results])
    vals = packed[:, :, :NCAND]
    idx = packed[:, :, NCAND:].astype(np.uint32)
    return vals, idx, res


def _postprocess(cls0, txty0, twth0, vals, idx):
    """Host epilogue. cls0 [80,256,256] f32, txty0/twth0 [2,256,256] f32,
    vals/idx [8,128,8]. Returns (bbox [100,4] f32, scores [100] f32, clses [100] i32).
    """
    core = np.arange(NCORES)[:, None, None]
    part = np.arange(128)[None, :, None]
    f = idx.reshape(-1).astype(np.int64)
    cls = (core * CPC + (idx // NBLK)).reshape(-1).astype(np.int64)
    j = (f % NBLK).astype(np.int64)
    p = np.broadcast_to(part, idx.shape).reshape(-1).astype(np.int64)

    # extracted 2x8 blocks -> 4 aligned 2x2 sub-blocks; only a sub-block
    # argmax can be a 5x5 peak (surviving peaks are >=3 apart)
    blk = cls0[
        cls[:, None, None],
        (2 * p)[:, None, None] + np.arange(2)[None, :, None],
        (BW_ * j)[:, None, None] + np.arange(BW_)[None, None, :],
    ]  # [N, 2, 8]
    N = blk.shape[0]
    sub = blk.reshape(N, 2, 4, 2).transpose(0, 2, 1, 3).reshape(N, 4, 4)  # [N,4,(2x2)]
    pos = sub.argmax(axis=2)  # [N, 4] in 0..3
    v = np.take_along_axis(sub, pos[:, :, None], axis=2)[:, :, 0]  # [N, 4]
    y = 2 * p[:, None] + pos // 2
    x = BW_ * j[:, None] + 2 * np.arange(4)[None, :] + pos % 2
    cls4 = np.broadcast_to(cls[:, None], (N, 4))
    v, y, x, cls4 = v.ravel(), y.ravel(), x.ravel(), cls4.ravel()

    # exact 5x5 peak test (clipped window == -inf padding for max)
    dy = np.arange(-2, 3)
    yy = np.clip(y[:, None] + dy[None, :], 0, H - 1)
    xx = np.clip(x[:, None] + dy[None, :], 0, W - 1)
    win = cls0[cls4[:, None, None], yy[:, :, None], xx[:, None, :]]
    is_peak = win.max(axis=(1, 2)) == v

    vv, cc, yy_, xx_ = v[is_peak], cls4[is_peak], y[is_peak], x[is_peak]
    order = np.argsort(-vv, kind="stable")[:TOPK]
    ok = order.size == TOPK
    if ok:
        vv, cc, yy_, xx_ = vv[order], cc[order], yy_[order], xx_[order]
        thr = vv[-1]
        # soundness: nothing outside the extracted blocks can reach the
        # top-100, and no extracted slot was lost to an exact value tie
        if (vals[:, :, -1] >= thr).any():
            ok = False
        else:
            srt = np.sort(idx, axis=2)
            dup = srt[:, :, 1:] == srt[:, :, :-1]
            if dup.any():
                dupv = np.minimum(vals[:, :, 1:], vals[:, :, :-1])[dup]
                if (dupv >= thr).any():
                    ok = False
    if not ok:  # pragma: no cover - never expected with continuous inputs
        return _reference_numpy(cls0, txty0, twth0)

    scores = (1.0 / (1.0 + np.exp(-vv))).astype(np.float32)
    clses = cc.astype(np.int32)

    sig = lambda a: (1.0 / (1.0 + np.exp(-a.astype(np.float32)))).astype(np.float32)
    cxp = (sig(txty0[0, yy_, xx_]) + xx_.astype(np.float32)) * STRIDE
    cyp = (sig(txty0[1, yy_, xx_]) + yy_.astype(np.float32)) * STRIDE
    wp = np.exp(twth0[0, yy_, xx_].astype(np.float32)) * STRIDE
    hp = np.exp(twth0[1, yy_, xx_].astype(np.float32)) * STRIDE
    scale = np.float32(H * STRIDE)
    bbox = (
        np.stack([cxp - wp * 0.5, cyp - hp * 0.5, cxp + wp * 0.5, cyp + hp * 0.5], -1)
        / scale
    )
    bbox = np.clip(bbox, 0.0, 1.0).astype(np.float32)
    return bbox, scores, clses


def _reference_numpy(cls0, txty0, twth0):
    """Exact safety-net implementation (numpy only); never expected to run."""
    prob = (1.0 / (1.0 + np.exp(-cls0.astype(np.float64)))).astype(np.float32)
    pad = np.full((prob.shape[0], H + 4, W + 4), -np.inf, np.float32)
    pad[:, 2:-2, 2:-2] = prob
    hmax = prob.copy()
    for ddy in range(5):
        for ddx in range(5):
            np.maximum(hmax, pad[:, ddy : ddy + H, ddx : ddx + W], out=hmax)
    masked = prob * (hmax == prob)
    C = masked.shape[0]
    flat = masked.reshape(C, H * W)
    i1 = np.argsort(-flat, axis=1, kind="stable")[:, :TOPK]
    s1 = np.take_along_axis(flat, i1, axis=1)
    s1f = s1.reshape(-1)
    i2 = np.argsort(-s1f, kind="stable")[:TOPK]
    clses = (i2 // TOPK).astype(np.int32)
    inds = i1.reshape(-1)[i2]
    yy_, xx_ = inds // W, inds % W
    scores = s1f[i2].astype(np.float32)
    sig = lambda a: (1.0 / (1.0 + np.exp(-a.astype(np.float32)))).astype(np.float32)
    cxp = (sig(txty0[0, yy_, xx_]) + xx_.astype(np.float32)) * STRIDE
    cyp = (sig(txty0[1, yy_, xx_]) + yy_.astype(np.float32)) * STRIDE
    wp = np.exp(twth0[0, yy_, xx_].astype(np.float32)) * STRIDE
    hp = np.exp(twth0[1, yy_, xx_].astype(np.float32)) * STRIDE
    scale = np.float32(H * STRIDE)
    bbox = (
        np.stack([cxp - wp * 0.5, cyp - hp * 0.5, cxp + wp * 0.5, cyp + hp * 0.5], -1)
        / scale
    )
    return np.clip(bbox, 0.0, 1.0).astype(np.float32), scores, clses


def kernel(cls_pred, txty_pred, twth_pred):
    cls0 = np.ascontiguousarray(np.asarray(cls_pred[0], dtype=np.float32))
    txty0 = np.asarray(txty_pred[0], dtype=np.float32)
    twth0 = np.asarray(twth_pred[0], dtype=np.float32)
    vals, idx, _ = _run_device(cls0)
    return _postprocess(cls0, txty0, twth0, vals, idx)
